# revision 32
# baseline (speedup 1.0000x reference)
"""Trainium2 Bass kernel for LocalAttentionLayer.

Problem: B=4, N=2048, H=8 heads, D=64, DM=512 (f32)
  q/k/v = x @ W{q,k,v}; sim = scale * q k^T (per head); mask_k/mask_q -> big_neg;
  softmax over keys; out = (attn @ v) @ Wo + bo.

Sharding (8 cores): core = 2*b + g -> batch b (4-way) x head-group g (2-way,
4 heads each).  Each core computes its batch's projections for its 4 heads,
full attention for those heads, and a partial output projection with its
256-row slice of Wo.  Host sums the two partials per batch, adds bo, and
overwrites masked-q rows (reference semantics: fully-masked rows degenerate
to uniform attention = mean over all v rows, computable on host as
(mean_j x) @ Wv @ Wo + bo).

Key optimizations over the naive layout:
  - Masked-position compaction: only kept q rows (Pq) and kept k rows (Pk)
    are shipped/computed; host gathers inputs and scatters outputs.  Pq/Pk
    are runtime values (q rounded up to 32, k to 128); one program is
    compiled per (Pq, Pk) and cached.
  - The attention inner loop is software-pipelined and ACT(Exp)-paced: sim
    j+2 is issued before pv j, each chunk's normalize is deferred until
    after the next chunk's first sims, and all non-attention PE work
    (k/q/v projections for later chunks, the hp1 projections, the output
    projection) is streamed through a work queue popped between sims so the
    Exp engine never waits at a phase boundary.
  - Softmax denominator rides along as a ones-column in v (col 64 of each
    68-wide head block), so P@V and the denominators come out of the same
    accumulation; all matmuls are bf16 (fp8 was measured too lossy: >1e-2).
  - PSUM-bank discipline: every matmul output stays inside one 2KB bank;
    the Exp uses strided 3-D APs to skip the inter-head alignment gap.
  - Copies and normalize run on DVE explicitly; ACT only does Exp.
  - Each input tensor loads as one wide multi-dim DMA (the HWDGE setup is
    a serial ~630ns/DMA resource), ordered by first use, with xq/xk split
    once so chunk-0 compute starts ~3us in.
"""

import sys

if "/opt/trn_rl_repo" not in sys.path:
    sys.path.insert(0, "/opt/trn_rl_repo")

from collections import deque

import os

import ml_dtypes
import numpy as np

SLACK_MARGIN = float(os.environ.get("K_SLACK", "274"))
LAG_LIMIT = int(os.environ.get("K_LAG", "6"))
PT_BUFS = int(os.environ.get("K_PTBUFS", "28"))

BF16 = np.dtype(ml_dtypes.bfloat16)

B, N, H, D = 4, 2048, 8, 64
DM = H * D  # 512
G = 2  # head-group split across cores
CG = DM // G  # 256 channels per group
HPG = H // G  # 4 heads per group
MASK_BIAS = -1.0e5
SHIFT = -4.0  # logit shift: keeps exp() comfortably in range without row max

_NC_CACHE = {}


def _build_nc(Pq, Pk):
    from contextlib import ExitStack

    import concourse.mybir as mybir
    import concourse.tile as tile
    from concourse import bacc
    from concourse.bass import ts

    f32 = mybir.dt.float32
    bf16 = mybir.dt.bfloat16
    EXP = mybir.ActivationFunctionType.Exp

    NJ = Pk // 128  # j-tiles
    # output i-tiles (last may be a partial tile: Pq is a multiple of 32)
    OT = []
    off = 0
    while off < Pq:
        OT.append((off, min(128, Pq - off)))
        off += 128
    NI = len(OT)
    # attention i-chunks: full 512-wide (sim output = whole PSUM bank per
    # head) plus a small remainder chunk.  hp0 visits the small chunk FIRST
    # (less DMA before the first Exp), hp1 visits it LAST (short drain after
    # the final Exp).
    CHS = [512] * (Pq // 512) + ([Pq % 512] if Pq % 512 else [])
    COFF = [sum(CHS[:i]) for i in range(len(CHS))]
    CHUNKS = list(zip(COFF, CHS))  # (i0, ic)
    ORDER = {
        0: sorted(CHUNKS, key=lambda t: t[1]),
        1: sorted(CHUNKS, key=lambda t: -t[1]),
    }
    ICK = Pk // 4  # k-projection chunk

    nc = bacc.Bacc(None, target_bir_lowering=False, debug=False)

    with tile.TileContext(nc) as tc, ExitStack() as ctx:
        dram = ctx.enter_context(tc.tile_pool(name="dram", bufs=1, space="DRAM"))
        const = ctx.enter_context(tc.tile_pool(name="const", bufs=1))
        ptp = ctx.enter_context(tc.tile_pool(name="ptp", bufs=PT_BUFS))
        fop = ctx.enter_context(tc.tile_pool(name="fop", bufs=4))
        rrp = ctx.enter_context(tc.tile_pool(name="rrp", bufs=2))
        psim = ctx.enter_context(tc.tile_pool(name="psim", bufs=2, space="PSUM"))
        ppv = ctx.enter_context(tc.tile_pool(name="ppv", bufs=1, space="PSUM"))
        pfo = ctx.enter_context(tc.tile_pool(name="pfo", bufs=2, space="PSUM"))

        # ---- DRAM I/O ----
        xqT_d = dram.tile([DM, Pq], bf16, kind="ExternalInput", name="xqT", uniquify=False)
        xkT_d = dram.tile([DM, Pk], bf16, kind="ExternalInput", name="xkT", uniquify=False)
        wq_d = dram.tile([DM, CG], bf16, kind="ExternalInput", name="wq", uniquify=False)
        wk_d = dram.tile([DM, CG], bf16, kind="ExternalInput", name="wk", uniquify=False)
        wva_d = dram.tile([DM, CG], bf16, kind="ExternalInput", name="wva", uniquify=False)
        wo_d = dram.tile([CG, DM], bf16, kind="ExternalInput", name="wo", uniquify=False)
        bk_d = dram.tile([128, NJ], f32, kind="ExternalInput", name="bk", uniquify=False)
        id_d = dram.tile([128, 128], bf16, kind="ExternalInput", name="ident", uniquify=False)
        out_d = dram.tile([Pq, DM], f32, kind="ExternalOutput", name="out", uniquify=False)

        # ---- SBUF persistents ----
        # The HWDGE + DMA engines are a serial resource (~630ns setup per
        # DMA), so each tensor loads as ONE wide DMA ([128, slices, cols]
        # APs), ordered by first use; xq/xk split once so chunk-0 arrives
        # early and compute starts ~3us in.
        xq_r = xqT_d.rearrange("(s p) i -> p s i", s=4, p=128)
        xk_r = xkT_d.rearrange("(s p) i -> p s i", s=4, p=128)
        wq_sb = const.tile([128, 4, CG], bf16, name="wq_sb")
        nc.sync.dma_start(out=wq_sb[:, :, :], in_=wq_d.rearrange("(s p) c -> p s c", s=4, p=128))
        xqT_sb = const.tile([128, 4, Pq], bf16, name="xqT_sb")
        nc.sync.dma_start(out=xqT_sb[:, :, 0 : CHS[0]], in_=xq_r[:, :, 0 : CHS[0]])
        wk_sb = const.tile([128, 4, CG], bf16, name="wk_sb")
        nc.sync.dma_start(out=wk_sb[:, :, :], in_=wk_d.rearrange("(s p) c -> p s c", s=4, p=128))
        bk_sb = const.tile_from(bk_d[:, :], name="bks")
        xkT_sb = const.tile([128, 4, Pk], bf16, name="xkT_sb")
        nc.sync.dma_start(out=xkT_sb[:, :, 0:ICK], in_=xk_r[:, :, 0:ICK])
        wva_sb = const.tile([128, 4, CG], bf16, name="wva_sb")
        nc.sync.dma_start(out=wva_sb[:, :, :], in_=wva_d.rearrange("(s p) c -> p s c", s=4, p=128))
        XKM = (ICK + Pk) // 2 // 32 * 32
        nc.sync.dma_start(out=xkT_sb[:, :, ICK:XKM], in_=xk_r[:, :, ICK:XKM])
        nc.sync.dma_start(out=xkT_sb[:, :, XKM:Pk], in_=xk_r[:, :, XKM:Pk])
        nc.sync.dma_start(out=xqT_sb[:, :, CHS[0] : Pq], in_=xq_r[:, :, CHS[0] : Pq])
        id_sb = const.tile([128, 128], bf16, name="id_sb")
        nc.sync.dma_start(out=id_sb[:, :], in_=id_d[:, :])
        wo_sb = const.tile([128, 2, DM], bf16, name="wo_sb")
        nc.sync.dma_start(out=wo_sb[:, :, :], in_=wo_d.rearrange("(s p) c -> p s c", s=2, p=128))



        qT_sb = [const.tile([128, Pq], bf16, name=f"qT{hp}") for hp in range(2)]
        kT_sb = [const.tile([128, Pk], bf16, name=f"kT{hp}") for hp in range(2)]
        aT_sb = [const.tile([128, Pq], bf16, name=f"aT{hp}") for hp in range(2)]
        # va: per j-tile [128, HPG*65] bf16: 4 heads x (64 v-cols + ones col);
        # the ones columns are memset once up front and never overwritten
        va_sb = [const.tile([128, HPG, 65], bf16, name=f"va{j}") for j in range(NJ)]
        for j in range(NJ):
            nc.vector.memset(va_sb[j][:, :, 64:65], 1.0)

        # ---- projection / output helpers ----
        def qk_proj_group(w_sb, x_sb, dst, hp, off, width):
            """One chunk of a q/k projection: dst[:, off:off+width]."""
            ps = pfo.tile([128, 512], f32, tag="fo", name="qk_ps")
            for k in range(4):
                nc.tensor.matmul(
                    ps[:, 0:width],
                    w_sb[:, k, hp * 128 : (hp + 1) * 128],
                    x_sb[:, k, off : off + width],
                    start=(k == 0),
                    stop=(k == 3),
                )
            nc.vector.tensor_copy(dst[:, off : off + width], ps[:, 0:width])

        # emitted-coverage bookkeeping: sims may only be emitted once the
        # kT/qT columns they read have their producers emitted (dependency
        # tracking follows emission order); cov counts contiguous columns
        cov = {"k0": 0, "k1": 0, "q0": [], "q1": []}

        def push_qk(w_sb, x_sb, dst, hp, off, width, kind):
            def f():
                qk_proj_group(w_sb, x_sb, dst, hp, off, width)
                if kind[0] == "k":
                    cov[kind] = max(cov[kind], off + width)
                else:
                    cov[kind].append((off, off + width))

            push(QK_NS, f)

        def force_k(hp, need):
            while cov[f"k{hp}"] < need:
                assert work_q, f"cannot extend k{hp} coverage to {need}"
                _pop_one()

        def force_q(hp, i0, ic):
            def done():
                return any(a <= i0 and i0 + ic <= b for a, b in cov[f"q{hp}"])

            while not done():
                assert work_q, f"cannot cover q{hp} [{i0}:{i0 + ic}]"
                _pop_one()

        def v_proj(j):
            v_ps = pfo.tile([128, CG], f32, tag="fo", name="v_ps")
            for k in range(4):
                nc.tensor.matmul(
                    v_ps[:, :],
                    xkT_sb[:, k, ts(j, 128)],
                    wva_sb[:, k, :],
                    start=(k == 0),
                    stop=(k == 3),
                )
            nc.vector.tensor_copy(
                va_sb[j][:, :, 0:64],
                v_ps.rearrange("p (h c) -> p h c", h=HPG, c=64)[:, :, :],
            )

        # ---- attention: globally slot-scheduled ----
        # ACT is the binding engine now.  Each j slot emits its sim matmuls
        # and the Exp; deferred PE units (k/q/v projections, chunk epilogues
        # with the two-pass pv + output projection) pop from a FIFO under a
        # per-slot cost budget so PE fills the ACT-bound slack.
        work_q = deque()  # (est_cost_ns, emit_fn, is_pv)
        budget = [0.0]
        pv_lag = [0]  # un-popped pv units; must stay under the pt ring depth
        PV_LAG_LIMIT = LAG_LIMIT

        def push(cost, fn, is_pv=False):
            work_q.append((cost, fn, is_pv))
            if is_pv:
                pv_lag[0] += 1

        def _pop_one():
            cost, fn, is_pv = work_q.popleft()
            budget[0] -= cost
            if is_pv:
                pv_lag[0] -= 1
            fn()

        def pops():
            while work_q and work_q[0][0] <= budget[0]:
                _pop_one()
            # a pv backlog deeper than the pt ring would deadlock the
            # in-order engines: force-drain ahead of budget
            while pv_lag[0] > PV_LAG_LIMIT:
                _pop_one()


        V_NS = 4 * CG * 0.42
        QK_NS = 4 * 512 * 0.42
        OUT_NS = (2 * DM + DM) * 0.42

        def mk_pvh(hp, h, io, ilen, i0, pts, a_sb, rr):
            """Two-pass pv for one (i-tile, head): replay the chunk's stored
            pt tiles as STATIONARY operands (ldweights are free) into a
            [128 q, 65] accumulator -- 65 output columns per j instead of ic,
            the softmax denominator riding along in column 64 -- then
            normalize with a per-partition reciprocal."""

            def f():
                epi_popped[0] += 1
                pvt = ppv.tile([128, 65], f32, tag=f"pv{h}", name=f"pv{h}")
                c0 = h * 512 + (io - i0)
                for j in range(NJ):
                    nc.tensor.matmul(
                        pvt[0:ilen, :],
                        pts[j][:, c0 : c0 + ilen],
                        va_sb[j][:, hp * 2 + h, :],
                        start=(j == 0),
                        stop=(j == NJ - 1),
                    )
                nc.vector.reciprocal(rr[0:ilen, h : h + 1], pvt[0:ilen, 64:65])
                nc.vector.tensor_scalar_mul(
                    a_sb[0:ilen, h * 64 : (h + 1) * 64],
                    pvt[0:ilen, 0:64],
                    rr[0:ilen, h : h + 1],
                )

            return f

        def mk_tx(hp, io, ilen, a_sb):
            """Flip the normalized [q, d] tile to [d, q] through the PE with
            an identity, then (hp1) the output projection for the i-tile."""

            def f():
                epi_popped[0] += 1
                tp = pfo.tile([128, 128], f32, tag="fo", name="tp")
                nc.tensor.matmul(
                    tp[:, 0:ilen],
                    a_sb[0:ilen, :],
                    id_sb[0:ilen, 0:ilen],
                    start=True,
                    stop=True,
                )
                nc.vector.tensor_copy(aT_sb[hp][:, io : io + ilen], tp[:, 0:ilen])
                if hp == 1:
                    out_proj_of(io, ilen)

            return f

        def out_proj_of(io, ilen):
            fo = pfo.tile([128, 512], f32, tag="fo", name="fo_ps")
            for c in range(2):
                nc.tensor.matmul(
                    fo[0:ilen, :],
                    aT_sb[c][:, io : io + ilen],
                    wo_sb[:, c, :],
                    start=(c == 0),
                    stop=(c == 1),
                )
            fo_sb = fop.tile([128, 512], f32, tag="fos", name="fo_sb")
            nc.vector.tensor_copy(fo_sb[0:ilen, :], fo[0:ilen, :])
            nc.sync.dma_start(out=out_d[io : io + ilen, :], in_=fo_sb[0:ilen, :])

        PVH_NS = NJ * 65 * 0.42
        TX_NS = (128 + 1024) * 0.42
        epi_pushed = [0]
        epi_popped = [0]
        chunk_marks = []  # epi_pushed watermark at each chunk start

        def attention(hp, i0, ic, last=False):
            # pt-ring safety: every epilogue reading pt tiles from two chunks
            # ago must be EMITTED before this chunk's sims reuse those ring
            # slots (dependency tracking follows emission order)
            chunk_marks.append(epi_pushed[0])
            if last:
                # the final chunk's own epilogue is the only drain after the
                # last Exp -- flush every earlier epilogue into this chunk's
                # slot stream instead of the tail
                while epi_popped[0] < epi_pushed[0]:
                    _pop_one()
            elif len(chunk_marks) >= 3:
                need = chunk_marks[-2]
                while epi_popped[0] < need:
                    _pop_one()
            slot_slack = 2 * ic * 0.4167 + SLACK_MARGIN
            if hp == 1:
                slot_slack += float(os.environ.get("K_SLACK1", "0"))

            def emit_sim(j):
                # head blocks bank-aligned at h*512: one matmul output window
                # per PSUM bank
                sm = psim.tile([128, 1024], f32, tag="sim", name="sm")
                for h in range(2):
                    hs = slice(h * 64, (h + 1) * 64)
                    nc.tensor.matmul(
                        sm[:, h * 512 : h * 512 + ic],
                        kT_sb[hp][hs, ts(j, 128)],
                        qT_sb[hp][hs, i0 : i0 + ic],
                        start=True,
                        stop=True,
                    )
                ptv = ptp.tile([128, 2 * 512], bf16, tag="pt", name="pt")
                sm_v = sm.rearrange("p (h c) -> p h c", h=2, c=512)[:, :, 0:ic]
                pt_v = ptv.rearrange("p (h c) -> p h c", h=2, c=512)[:, :, 0:ic]
                nc.scalar.activation(pt_v, sm_v, EXP, bias=bk_sb[:, j : j + 1], scale=1.0)
                return ptv

            force_q(hp, i0, ic)
            force_k(hp, 128)
            pts = [emit_sim(0)]
            for j in range(NJ):
                if j + 1 < NJ:
                    force_k(hp, min((j + 2) * 128, Pk))
                    pts.append(emit_sim(j + 1))
                if hp == 0 and i0 == ORDER[0][0][0] and j + 2 < NJ:
                    push(V_NS, lambda j2=j + 2: v_proj(j2))
                budget[0] += slot_slack
                pops()
            # chunk epilogue units (consume this chunk's pt ring slots); they
            # pop inside the following chunks' slot slack.  is_pv bounds how
            # far they may lag so the pt ring never wraps onto unread tiles.
            off = i0
            while off < i0 + ic:
                ilen = min(128, i0 + ic - off)
                a_sb = rrp.tile([128, 128], bf16, tag="an", name="an")
                rr = rrp.tile([128, 2], f32, tag="rr", name="rr")
                for h in range(2):
                    push(PVH_NS, mk_pvh(hp, h, off, ilen, i0, pts, a_sb, rr), is_pv=True)
                    epi_pushed[0] += 1
                push(TX_NS, mk_tx(hp, off, ilen, a_sb), is_pv=True)
                epi_pushed[0] += 1
                off += ilen

        # ---- program ----
        first0 = ORDER[0][0]
        qk_proj_group(wq_sb, xqT_sb, qT_sb[0], 0, first0[0], first0[1])
        cov["q0"].append((first0[0], first0[0] + first0[1]))
        qk_proj_group(wk_sb, xkT_sb, kT_sb[0], 0, 0, ICK)
        cov["k0"] = ICK
        v_proj(0)
        v_proj(1)

        # seed deferred work: k/q chunks first (hard deadlines: k for this
        # chunk's own sims, q before chunk 1), v next (needed only when the
        # corresponding pv pops, well after slot j)
        for ch in range(1, 4):
            push_qk(wk_sb, xkT_sb, kT_sb[0], 0, ch * ICK, ICK, "k0")
        for i0c, icc in ORDER[0][1:]:
            push_qk(wq_sb, xqT_sb, qT_sb[0], 0, i0c, icc, "q0")
        for hp in range(2):
            for ci, (i0c, icc) in enumerate(ORDER[hp]):
                if (hp, ci) == (0, 1):
                    # hp1 projections: pushed here so they pop before hp1
                    for i2, c2 in ORDER[1]:
                        push_qk(wq_sb, xqT_sb, qT_sb[1], 1, i2, c2, "q1")
                    for c2 in range(4):
                        push_qk(wk_sb, xkT_sb, kT_sb[1], 1, c2 * ICK, ICK, "k1")
                attention(hp, i0c, icc, last=(hp == 1 and ci == len(ORDER[1]) - 1))
        budget[0] = 1e12
        pops()

    nc.compile()
    return nc


def _get_nc(Pq=1696, Pk=1664):
    key = (Pq, Pk)
    if key not in _NC_CACHE:
        _NC_CACHE[key] = _build_nc(Pq, Pk)
    return _NC_CACHE[key]


def _roundup(n, m):
    return ((n + m - 1) // m) * m


def kernel(x, Wq, Wk, Wv, Wo, bo, mask_k, mask_q):
    from concourse import bass_utils

    x = np.asarray(x, np.float32)
    Wq = np.asarray(Wq, np.float32)
    Wk = np.asarray(Wk, np.float32)
    Wv = np.asarray(Wv, np.float32)
    Wo = np.asarray(Wo, np.float32)
    bo = np.asarray(bo, np.float32)
    mask_k = np.asarray(mask_k)
    mask_q = np.asarray(mask_q)

    qidx = [np.nonzero(mask_q[b])[0] for b in range(B)]
    kidx = [np.nonzero(mask_k[b])[0] for b in range(B)]
    # compaction pads; >=1024 keeps the chunk layout simple
    Pq = max(_roundup(max(len(i) for i in qidx), 32), 1024)
    Pk = max(_roundup(max(len(i) for i in kidx), 128), 1024)
    NJ = Pk // 128

    nc = _get_nc(Pq, Pk)
    scale = float(D) ** -0.5

    in_maps = []
    for core in range(8):
        b, g = core // 2, core % 2
        cs = slice(g * CG, (g + 1) * CG)
        qi, ki = qidx[b], kidx[b]
        xq = np.zeros((Pq, DM), np.float32)
        xq[: len(qi)] = x[b][qi]
        xk = np.zeros((Pk, DM), np.float32)
        xk[: len(ki)] = x[b][ki]
        bk = np.full((Pk,), MASK_BIAS, np.float32)
        bk[: len(ki)] = SHIFT
        in_maps.append(
            {
                "xqT": np.ascontiguousarray(xq.T).astype(BF16),
                "xkT": np.ascontiguousarray(xk.T).astype(BF16),
                "wq": np.ascontiguousarray(Wq[:, cs] * scale).astype(BF16),
                "wk": np.ascontiguousarray(Wk[:, cs]).astype(BF16),
                "wva": np.ascontiguousarray(Wv[:, cs]).astype(BF16),
                "wo": np.ascontiguousarray(Wo[cs, :]).astype(BF16),
                "bk": np.ascontiguousarray(bk.reshape(NJ, 128).T),
                "ident": np.eye(128, dtype=np.float32).astype(BF16),
            }
        )

    global _LAST_IN_MAPS, _LAST_NC
    _LAST_IN_MAPS = in_maps
    _LAST_NC = nc
    res = bass_utils.run_bass_kernel_spmd(nc, in_maps, core_ids=list(range(8)))
    outs = res.results

    out = np.empty((B, N, DM), np.float32)
    for b in range(B):
        o = outs[2 * b]["out"] + outs[2 * b + 1]["out"]
        full = np.empty((N, DM), np.float32)
        full[qidx[b]] = o[: len(qidx[b])] + bo[None, :]
        # reference semantics for fully-masked query rows: uniform attention
        uf = (x[b].mean(0) @ Wv) @ Wo + bo
        full[~mask_q[b]] = uf
        out[b] = full
    return out



# revision 46
# speedup vs baseline: 1.1884x; 1.1884x over previous
"""Trainium2 Bass kernel for LocalAttentionLayer.

Problem: B=4, N=2048, H=8 heads, D=64, DM=512 (f32)
  q/k/v = x @ W{q,k,v}; sim = scale * q k^T (per head); mask_k/mask_q -> big_neg;
  softmax over keys; out = (attn @ v) @ Wo + bo.

Sharding (8 cores): core = 2*b + g -> batch b (4-way) x head-group g (2-way,
4 heads each).  Each core computes its batch's projections for its 4 heads,
full attention for those heads, and a partial output projection with its
256-row slice of Wo.  Host sums the two partials per batch, adds bo, and
overwrites masked-q rows (reference semantics: fully-masked rows degenerate
to uniform attention = mean over all v rows, computable on host as
(mean_j x) @ Wv @ Wo + bo).

Key optimizations over the naive layout:
  - Masked-position compaction: only kept q rows (Pq) and kept k rows (Pk)
    are shipped/computed; host gathers inputs and scatters outputs.  Pq/Pk
    are runtime values (q rounded up to 32, k to 128); one program is
    compiled per (Pq, Pk) and cached.
  - The attention inner loop is software-pipelined and ACT(Exp)-paced: sim
    j+2 is issued before pv j, each chunk's normalize is deferred until
    after the next chunk's first sims, and all non-attention PE work
    (k/q/v projections for later chunks, the hp1 projections, the output
    projection) is streamed through a work queue popped between sims so the
    Exp engine never waits at a phase boundary.
  - Softmax denominator rides along as a ones-column in v (col 64 of each
    68-wide head block), so P@V and the denominators come out of the same
    accumulation; all matmuls are bf16 (fp8 was measured too lossy: >1e-2).
  - PSUM-bank discipline: every matmul output stays inside one 2KB bank;
    the Exp uses strided 3-D APs to skip the inter-head alignment gap.
  - Copies and normalize run on DVE explicitly; ACT only does Exp.
  - Each input tensor loads as one wide multi-dim DMA (the HWDGE setup is
    a serial ~630ns/DMA resource), ordered by first use, with xq/xk split
    once so chunk-0 compute starts ~3us in.
"""

import sys

if "/opt/trn_rl_repo" not in sys.path:
    sys.path.insert(0, "/opt/trn_rl_repo")

from collections import deque

import os

import ml_dtypes
import numpy as np

SLACK_MARGIN = float(os.environ.get("K_SLACK", "120"))
LAG_LIMIT = int(os.environ.get("K_LAG", "6"))
PT_BUFS = int(os.environ.get("K_PTBUFS", "28"))

BF16 = np.dtype(ml_dtypes.bfloat16)

B, N, H, D = 4, 2048, 8, 64
DM = H * D  # 512
G = 2  # head-group split across cores
CG = DM // G  # 256 channels per group
HPG = H // G  # 4 heads per group
MASK_BIAS = -1.0e5
SHIFT = -4.0  # logit shift: keeps exp() comfortably in range without row max

_NC_CACHE = {}


def _build_nc(Pq, Pk):
    from contextlib import ExitStack

    import concourse.mybir as mybir
    import concourse.tile as tile
    from concourse import bacc
    from concourse.bass import ts

    f32 = mybir.dt.float32
    bf16 = mybir.dt.bfloat16
    EXP = mybir.ActivationFunctionType.Exp

    NJ = Pk // 128  # j-tiles
    # output i-tiles (last may be a partial tile: Pq is a multiple of 32)
    OT = []
    off = 0
    while off < Pq:
        OT.append((off, min(128, Pq - off)))
        off += 128
    NI = len(OT)
    # attention i-chunks: full 512-wide (sim output = whole PSUM bank per
    # head) plus a small remainder chunk.  hp0 visits the small chunk FIRST
    # (less DMA before the first Exp), hp1 visits it LAST (short drain after
    # the final Exp).
    CHS = [512] * (Pq // 512) + ([Pq % 512] if Pq % 512 else [])
    COFF = [sum(CHS[:i]) for i in range(len(CHS))]
    CHUNKS = list(zip(COFF, CHS))  # (i0, ic)
    if int(os.environ.get("K_ORDER", "0")):
        ORDER = {
            0: sorted(CHUNKS, key=lambda t: t[1]),
            1: sorted(CHUNKS, key=lambda t: -t[1]),
        }
    else:
        ORDER = {0: list(CHUNKS), 1: list(CHUNKS)}
    ICK = min(512, Pk)  # first k-projection chunk (one PSUM bank wide)
    K_CHUNKS = []
    _o = ICK
    while _o < Pk:
        K_CHUNKS.append((_o, min(512, Pk - _o)))
        _o += 512

    nc = bacc.Bacc(None, target_bir_lowering=False, debug=False)

    with tile.TileContext(nc) as tc, ExitStack() as ctx:
        dram = ctx.enter_context(tc.tile_pool(name="dram", bufs=1, space="DRAM"))
        const = ctx.enter_context(tc.tile_pool(name="const", bufs=1))
        ptp = ctx.enter_context(tc.tile_pool(name="ptp", bufs=PT_BUFS))
        fop = ctx.enter_context(tc.tile_pool(name="fop", bufs=4))
        rrp = ctx.enter_context(tc.tile_pool(name="rrp", bufs=6))
        psim = ctx.enter_context(tc.tile_pool(name="psim", bufs=2, space="PSUM"))
        ppv = ctx.enter_context(tc.tile_pool(name="ppv", bufs=1, space="PSUM"))
        pfo = ctx.enter_context(tc.tile_pool(name="pfo", bufs=2, space="PSUM"))

        # ---- DRAM I/O ----
        xqT_d = dram.tile([DM, Pq], bf16, kind="ExternalInput", name="xqT", uniquify=False)
        xkT_d = dram.tile([DM, Pk], bf16, kind="ExternalInput", name="xkT", uniquify=False)
        wq_d = dram.tile([DM, CG], bf16, kind="ExternalInput", name="wq", uniquify=False)
        wk_d = dram.tile([DM, CG], bf16, kind="ExternalInput", name="wk", uniquify=False)
        wva_d = dram.tile([DM, CG], bf16, kind="ExternalInput", name="wva", uniquify=False)
        wo_d = dram.tile([CG, DM], bf16, kind="ExternalInput", name="wo", uniquify=False)
        bk_d = dram.tile([128, NJ], f32, kind="ExternalInput", name="bk", uniquify=False)
        id_d = dram.tile([128, 128], bf16, kind="ExternalInput", name="ident", uniquify=False)
        out_d = dram.tile([Pq, DM], bf16, kind="ExternalOutput", name="out", uniquify=False)

        # ---- SBUF persistents ----
        # The HWDGE + DMA engines are a serial resource (~630ns setup per
        # DMA), so each tensor loads as ONE wide DMA ([128, slices, cols]
        # APs), ordered by first use; xq/xk split once so chunk-0 arrives
        # early and compute starts ~3us in.
        xq_r = xqT_d.rearrange("(s p) i -> p s i", s=4, p=128)
        xk_r = xkT_d.rearrange("(s p) i -> p s i", s=4, p=128)
        wq_sb = const.tile([128, 4, CG], bf16, name="wq_sb")
        nc.sync.dma_start(out=wq_sb[:, :, :], in_=wq_d.rearrange("(s p) c -> p s c", s=4, p=128))
        xqT_sb = const.tile([128, 4, Pq], bf16, name="xqT_sb")
        nc.sync.dma_start(out=xqT_sb[:, :, 0 : CHS[0]], in_=xq_r[:, :, 0 : CHS[0]])
        wk_sb = const.tile([128, 4, CG], bf16, name="wk_sb")
        nc.sync.dma_start(out=wk_sb[:, :, :], in_=wk_d.rearrange("(s p) c -> p s c", s=4, p=128))
        bk_sb = const.tile_from(bk_d[:, :], name="bks")
        xkT_sb = const.tile([128, 4, Pk], bf16, name="xkT_sb")
        XK0 = min(256, ICK)
        nc.sync.dma_start(out=xkT_sb[:, :, 0:XK0], in_=xk_r[:, :, 0:XK0])
        nc.sync.dma_start(out=xkT_sb[:, :, XK0:ICK], in_=xk_r[:, :, XK0:ICK])
        nc.sync.dma_start(out=xkT_sb[:, :, ICK:Pk], in_=xk_r[:, :, ICK:Pk])
        wva_sb = const.tile([128, 4, CG], bf16, name="wva_sb")
        nc.sync.dma_start(out=wva_sb[:, :, :], in_=wva_d.rearrange("(s p) c -> p s c", s=4, p=128))
        nc.sync.dma_start(out=xqT_sb[:, :, CHS[0] : Pq], in_=xq_r[:, :, CHS[0] : Pq])
        id_sb = const.tile([128, 128], bf16, name="id_sb")
        nc.sync.dma_start(out=id_sb[:, :], in_=id_d[:, :])
        wo_sb = const.tile([128, 2, DM], bf16, name="wo_sb")
        nc.sync.dma_start(out=wo_sb[:, :, :], in_=wo_d.rearrange("(s p) c -> p s c", s=2, p=128))



        qT_sb = [const.tile([128, Pq], bf16, name=f"qT{hp}") for hp in range(2)]
        kT_sb = [const.tile([128, Pk], bf16, name=f"kT{hp}") for hp in range(2)]
        aT_sb = [const.tile([128, Pq], bf16, name=f"aT{hp}") for hp in range(2)]
        # va: per j-tile [128, HPG*65] bf16: 4 heads x (64 v-cols + ones col);
        # the ones columns are memset once up front and never overwritten
        va_sb = [const.tile([128, HPG, 65], bf16, name=f"va{j}") for j in range(NJ)]
        for j in range(NJ):
            nc.vector.memset(va_sb[j][:, :, 64:65], 1.0)
        if int(os.environ.get("K_DUMMY", "1")):
            # dummy Exp: pulls the ~1.3us ACT_TABLE_LOAD to t=0 instead of
            # just before the first real Exp
            dum = const.tile([1, 2], f32, name="dum")
            nc.vector.memset(dum[:, :], 0.0)
            nc.scalar.activation(dum[0:1, 1:2], dum[0:1, 0:1], EXP)

        # ---- projection / output helpers ----
        def qk_proj_group(w_sb, x_sb, dst, hp, off, width):
            """One chunk of a q/k projection: dst[:, off:off+width]."""
            ps = pfo.tile([128, 512], f32, tag="fo", name="qk_ps")
            for k in range(4):
                nc.tensor.matmul(
                    ps[:, 0:width],
                    w_sb[:, k, hp * 128 : (hp + 1) * 128],
                    x_sb[:, k, off : off + width],
                    start=(k == 0),
                    stop=(k == 3),
                )
            nc.vector.tensor_copy(dst[:, off : off + width], ps[:, 0:width])

        # emitted-coverage bookkeeping: sims may only be emitted once the
        # kT/qT columns they read have their producers emitted (dependency
        # tracking follows emission order); cov counts contiguous columns
        cov = {"k0": 0, "k1": 0, "q0": [], "q1": []}

        NSLOT_HP = len(CHS) * NJ  # slots per head-pair

        def push_qk(w_sb, x_sb, dst, hp, off, width, kind, deadline=10**9):
            def f():
                qk_proj_group(w_sb, x_sb, dst, hp, off, width)
                if kind[0] == "k":
                    cov[kind] = max(cov[kind], off + width)
                else:
                    cov[kind].append((off, off + width))

            push(QK_NS, f, deadline=deadline)

        def force_k(hp, need):
            while cov[f"k{hp}"] < need:
                assert work_q, f"cannot extend k{hp} coverage to {need}"
                _pop_one()

        def force_q(hp, i0, ic):
            def done():
                return any(a <= i0 and i0 + ic <= b for a, b in cov[f"q{hp}"])

            while not done():
                assert work_q, f"cannot cover q{hp} [{i0}:{i0 + ic}]"
                _pop_one()

        def v_proj(j):
            v_ps = pfo.tile([128, CG], f32, tag="fo", name="v_ps")
            for k in range(4):
                nc.tensor.matmul(
                    v_ps[:, :],
                    xkT_sb[:, k, ts(j, 128)],
                    wva_sb[:, k, :],
                    start=(k == 0),
                    stop=(k == 3),
                )
            nc.vector.tensor_copy(
                va_sb[j][:, :, 0:64],
                v_ps.rearrange("p (h c) -> p h c", h=HPG, c=64)[:, :, :],
            )

        # ---- attention: globally slot-scheduled ----
        # ACT is the binding engine now.  Each j slot emits its sim matmuls
        # and the Exp; deferred PE units (k/q/v projections, chunk epilogues
        # with the two-pass pv + output projection) pop from a FIFO under a
        # per-slot cost budget so PE fills the ACT-bound slack.
        work_q = deque()  # (est_cost_ns, emit_fn, is_pv, deadline_slot)
        budget = [0.0]
        pv_lag = [0]  # un-popped epilogue units; bounded by the pt ring depth
        PV_LAG_LIMIT = LAG_LIMIT
        cur_slot = [0]
        LOOKAHEAD = int(os.environ.get("K_LOOKAHEAD", "3"))

        def push(cost, fn, is_pv=False, deadline=10**9):
            work_q.append((cost, fn, is_pv, deadline))
            if is_pv:
                pv_lag[0] += 1

        def _pop_item(item):
            cost, fn, is_pv, _ = item
            budget[0] -= cost
            if is_pv:
                pv_lag[0] -= 1
            fn()

        def _pop_one():
            _pop_item(work_q.popleft())

        def pops():
            # deadline-near units pop regardless of budget, spread across
            # slots instead of bursting at a chunk boundary
            horizon = cur_slot[0] + LOOKAHEAD
            while True:
                hit = next((i for i, u in enumerate(work_q) if u[3] <= horizon), None)
                if hit is None:
                    break
                item = work_q[hit]
                del work_q[hit]
                _pop_item(item)
            while work_q and work_q[0][0] <= budget[0]:
                _pop_one()
            # an epilogue backlog deeper than the pt ring would corrupt the
            # ring: force-drain ahead of budget
            while pv_lag[0] > PV_LAG_LIMIT:
                _pop_one()


        V_NS = 4 * CG * 0.42
        QK_NS = 4 * 512 * 0.42
        OUT_NS = (2 * DM + DM) * 0.42

        held_pv = {}

        def mk_pv_partial(hp, h, io, ilen, i0, pts, js):
            def f():
                epi_popped[0] += 1
                pvt = ppv.tile([128, 65], f32, tag=f"pv{h}", name=f"pv{h}")
                held_pv[(io, h)] = pvt
                c0 = h * 512 + (io - i0)
                for j in range(js):
                    nc.tensor.matmul(
                        pvt[0:ilen, :],
                        pts[j][:, c0 : c0 + ilen],
                        va_sb[j][:, hp * 2 + h, :],
                        start=(j == 0),
                        stop=False,
                    )

            return f

        def mk_pv_finish(hp, h, io, ilen, i0, pts, js, a_sb, rr):
            def f():
                epi_popped[0] += 1
                pvt = held_pv.pop((io, h))
                c0 = h * 512 + (io - i0)
                for j in range(js, NJ):
                    nc.tensor.matmul(
                        pvt[0:ilen, :],
                        pts[j][:, c0 : c0 + ilen],
                        va_sb[j][:, hp * 2 + h, :],
                        start=False,
                        stop=(j == NJ - 1),
                    )
                nc.vector.reciprocal(rr[0:ilen, h : h + 1], pvt[0:ilen, 64:65])
                nc.vector.tensor_scalar_mul(
                    a_sb[0:ilen, h * 64 : (h + 1) * 64],
                    pvt[0:ilen, 0:64],
                    rr[0:ilen, h : h + 1],
                )

            return f

        def mk_pvh(hp, h, io, ilen, i0, pts, a_sb, rr):
            """Two-pass pv for one (i-tile, head): replay the chunk's stored
            pt tiles as STATIONARY operands (ldweights are free) into a
            [128 q, 65] accumulator -- 65 output columns per j instead of ic,
            the softmax denominator riding along in column 64 -- then
            normalize with a per-partition reciprocal."""

            def f():
                epi_popped[0] += 1
                pvt = ppv.tile([128, 65], f32, tag=f"pv{h}", name=f"pv{h}")
                c0 = h * 512 + (io - i0)
                for j in range(NJ):
                    nc.tensor.matmul(
                        pvt[0:ilen, :],
                        pts[j][:, c0 : c0 + ilen],
                        va_sb[j][:, hp * 2 + h, :],
                        start=(j == 0),
                        stop=(j == NJ - 1),
                    )
                nc.vector.reciprocal(rr[0:ilen, h : h + 1], pvt[0:ilen, 64:65])
                nc.vector.tensor_scalar_mul(
                    a_sb[0:ilen, h * 64 : (h + 1) * 64],
                    pvt[0:ilen, 0:64],
                    rr[0:ilen, h : h + 1],
                )

            return f

        def mk_tx(hp, io, ilen, a_sb):
            """Flip the normalized [q, d] tile to [d, q] through the PE with
            an identity, then (hp1) the output projection for the i-tile."""

            def f():
                epi_popped[0] += 1
                tp = pfo.tile([128, 128], f32, tag="fo", name="tp")
                nc.tensor.matmul(
                    tp[:, 0:ilen],
                    a_sb[0:ilen, :],
                    id_sb[0:ilen, 0:ilen],
                    start=True,
                    stop=True,
                )
                nc.vector.tensor_copy(aT_sb[hp][:, io : io + ilen], tp[:, 0:ilen])
                if hp == 1:
                    out_proj_of(io, ilen)

            return f

        def out_proj_of(io, ilen):
            fo = pfo.tile([128, 512], f32, tag="fo", name="fo_ps")
            for c in range(2):
                nc.tensor.matmul(
                    fo[0:ilen, :],
                    aT_sb[c][:, io : io + ilen],
                    wo_sb[:, c, :],
                    start=(c == 0),
                    stop=(c == 1),
                )
            fo_sb = fop.tile([128, 512], bf16, tag="fos", name="fo_sb")
            nc.vector.tensor_copy(fo_sb[0:ilen, :], fo[0:ilen, :])
            nc.sync.dma_start(out=out_d[io : io + ilen, :], in_=fo_sb[0:ilen, :])

        PVH_NS = NJ * 65 * 0.42
        TX_NS = (128 + 1024) * 0.42
        epi_pushed = [0]
        epi_popped = [0]
        chunk_marks = []  # epi_pushed watermark at each chunk start

        def attention(hp, i0, ic, last=False):
            # pt-ring safety: every epilogue reading pt tiles from two chunks
            # ago must be EMITTED before this chunk's sims reuse those ring
            # slots (dependency tracking follows emission order)
            chunk_marks.append(epi_pushed[0])
            if last and int(os.environ.get("K_DRAIN", "1")):
                # the final chunk's own epilogue is the only drain after the
                # last Exp -- flush every earlier epilogue into this chunk's
                # slot stream instead of the tail
                while epi_popped[0] < epi_pushed[0]:
                    _pop_one()
            elif len(chunk_marks) >= 3:
                need = chunk_marks[-2]
                while epi_popped[0] < need:
                    _pop_one()
            slot_slack = 2 * ic * 0.4167 + SLACK_MARGIN
            if hp == 1:
                slot_slack += float(os.environ.get("K_SLACK1", "0"))

            def emit_sim(j):
                # head blocks bank-aligned at h*512: one matmul output window
                # per PSUM bank
                sm = psim.tile([128, 1024], f32, tag="sim", name="sm")
                for h in range(2):
                    hs = slice(h * 64, (h + 1) * 64)
                    nc.tensor.matmul(
                        sm[:, h * 512 : h * 512 + ic],
                        kT_sb[hp][hs, ts(j, 128)],
                        qT_sb[hp][hs, i0 : i0 + ic],
                        start=True,
                        stop=True,
                    )
                ptv = ptp.tile([128, 2 * 512], bf16, tag="pt", name="pt")
                sm_v = sm.rearrange("p (h c) -> p h c", h=2, c=512)[:, :, 0:ic]
                pt_v = ptv.rearrange("p (h c) -> p h c", h=2, c=512)[:, :, 0:ic]
                nc.scalar.activation(pt_v, sm_v, EXP, bias=bk_sb[:, j : j + 1], scale=1.0)
                return ptv

            force_q(hp, i0, ic)
            force_k(hp, 128)
            JS = NJ - 2
            pts = [emit_sim(0)]
            for j in range(NJ):
                cur_slot[0] += 1
                if j + 1 < NJ:
                    force_k(hp, min((j + 2) * 128, Pk))
                    pts.append(emit_sim(j + 1))
                if hp == 0 and i0 == ORDER[0][0][0] and j + 2 < NJ:
                    push(V_NS, lambda j2=j + 2: v_proj(j2))
                if last and j == JS and ic >= 128:
                    # head-start the final i-tile's pv so only two j-tiles
                    # remain after the last Exp
                    for h in range(2):
                        push(JS * 65 * 0.42, mk_pv_partial(hp, h, i0, 128, i0, pts, JS), is_pv=True)
                        epi_pushed[0] += 1
                budget[0] += slot_slack
                pops()
            # chunk epilogue units (consume this chunk's pt ring slots); they
            # pop inside the following chunks' slot slack.  is_pv bounds how
            # far they may lag so the pt ring never wraps onto unread tiles.
            off = i0
            COARSE = int(os.environ.get("K_COARSE", "1"))
            while off < i0 + ic:
                ilen = min(128, i0 + ic - off)
                a_sb = rrp.tile([128, 128], bf16, tag="an", name="an")
                rr = rrp.tile([128, 2], f32, tag="rr", name="rr")
                if last and off == i0 and ic >= 128:
                    # finish the head-started tile: 2 j-tiles + normalize + tx
                    def fin(hp=hp, off=off, i0=i0, a_sb=a_sb, rr=rr, pts=pts, JS=JS):
                        # inner calls bump epi_popped once each (3 total);
                        # net for this single pushed unit must be +1 -> -2
                        epi_popped[0] -= 2
                        mk_pv_finish(hp, 0, off, 128, i0, pts, JS, a_sb, rr)()
                        mk_pv_finish(hp, 1, off, 128, i0, pts, JS, a_sb, rr)()
                        mk_tx(hp, off, 128, a_sb)()

                    push((2 * (NJ - JS) * 65 + 128 + 1024) * 0.42, fin, is_pv=True)
                    epi_pushed[0] += 1
                    off += 128
                    continue
                if COARSE:
                    u0 = mk_pvh(hp, 0, off, ilen, i0, pts, a_sb, rr)
                    u1 = mk_pvh(hp, 1, off, ilen, i0, pts, a_sb, rr)
                    u2 = mk_tx(hp, off, ilen, a_sb)

                    def fused(u0=u0, u1=u1, u2=u2):
                        u0(), u1(), u2()
                        epi_popped[0] += 2  # fused units count as three pops

                    push(2 * PVH_NS + TX_NS, fused, is_pv=True)
                    epi_pushed[0] += 1
                else:
                    for h in range(2):
                        push(PVH_NS, mk_pvh(hp, h, off, ilen, i0, pts, a_sb, rr), is_pv=True)
                        epi_pushed[0] += 1
                    push(TX_NS, mk_tx(hp, off, ilen, a_sb), is_pv=True)
                    epi_pushed[0] += 1
                off += ilen

        # ---- program ----
        first0 = ORDER[0][0]
        qk_proj_group(wq_sb, xqT_sb, qT_sb[0], 0, first0[0], first0[1])
        cov["q0"].append((first0[0], first0[0] + first0[1]))
        qk_proj_group(wk_sb, xkT_sb, kT_sb[0], 0, 0, XK0)
        qk_proj_group(wk_sb, xkT_sb, kT_sb[0], 0, XK0, ICK - XK0)
        cov["k0"] = ICK
        push(V_NS, lambda: v_proj(0))
        push(V_NS, lambda: v_proj(1))

        # seed deferred work: k/q chunks first (hard deadlines: k for this
        # chunk's own sims, q before chunk 1), v next (needed only when the
        # corresponding pv pops, well after slot j)
        for a, w in K_CHUNKS:
            # k columns [a:b] are read by slot a//128 of every chunk; the
            # binding one is the first chunk of the head-pair
            push_qk(wk_sb, xkT_sb, kT_sb[0], 0, a, w, "k0",
                    deadline=max(0, a // 128 - 1))
        for ni, (i0c, icc) in enumerate(ORDER[0][1:]):
            push_qk(wq_sb, xqT_sb, qT_sb[0], 0, i0c, icc, "q0",
                    deadline=(ni + 1) * NJ - 1)
        for hp in range(2):
            for ci, (i0c, icc) in enumerate(ORDER[hp]):
                if (hp, ci) == (0, 1):
                    # hp1 projections: pushed here so they pop before hp1
                    for ni, (i2, c2) in enumerate(ORDER[1]):
                        push_qk(wq_sb, xqT_sb, qT_sb[1], 1, i2, c2, "q1",
                                deadline=NSLOT_HP + ni * NJ - 1)
                    for a, w in [(0, ICK)] + K_CHUNKS:
                        push_qk(wk_sb, xkT_sb, kT_sb[1], 1, a, w, "k1",
                                deadline=NSLOT_HP + max(0, a // 128 - 1))
                attention(hp, i0c, icc, last=(hp == 1 and ci == len(ORDER[1]) - 1))
        budget[0] = 1e12
        pops()
        while work_q:
            _pop_one()

    nc.compile()
    return nc


def _get_nc(Pq=1696, Pk=1664):
    key = (Pq, Pk)
    if key not in _NC_CACHE:
        _NC_CACHE[key] = _build_nc(Pq, Pk)
    return _NC_CACHE[key]


def _roundup(n, m):
    return ((n + m - 1) // m) * m


def kernel(x, Wq, Wk, Wv, Wo, bo, mask_k, mask_q):
    from concourse import bass_utils

    x = np.asarray(x, np.float32)
    Wq = np.asarray(Wq, np.float32)
    Wk = np.asarray(Wk, np.float32)
    Wv = np.asarray(Wv, np.float32)
    Wo = np.asarray(Wo, np.float32)
    bo = np.asarray(bo, np.float32)
    mask_k = np.asarray(mask_k)
    mask_q = np.asarray(mask_q)

    qidx = [np.nonzero(mask_q[b])[0] for b in range(B)]
    kidx = [np.nonzero(mask_k[b])[0] for b in range(B)]
    # compaction pads; >=1024 keeps the chunk layout simple
    Pq = max(_roundup(max(len(i) for i in qidx), 32), 1024)
    Pk = max(_roundup(max(len(i) for i in kidx), 128), 1024)
    NJ = Pk // 128

    nc = _get_nc(Pq, Pk)
    scale = float(D) ** -0.5

    in_maps = []
    for core in range(8):
        b, g = core // 2, core % 2
        cs = slice(g * CG, (g + 1) * CG)
        qi, ki = qidx[b], kidx[b]
        xq = np.zeros((Pq, DM), np.float32)
        xq[: len(qi)] = x[b][qi]
        xk = np.zeros((Pk, DM), np.float32)
        xk[: len(ki)] = x[b][ki]
        bk = np.full((Pk,), MASK_BIAS, np.float32)
        bk[: len(ki)] = SHIFT
        in_maps.append(
            {
                "xqT": np.ascontiguousarray(xq.T).astype(BF16),
                "xkT": np.ascontiguousarray(xk.T).astype(BF16),
                "wq": np.ascontiguousarray(Wq[:, cs] * scale).astype(BF16),
                "wk": np.ascontiguousarray(Wk[:, cs]).astype(BF16),
                "wva": np.ascontiguousarray(Wv[:, cs]).astype(BF16),
                "wo": np.ascontiguousarray(Wo[cs, :]).astype(BF16),
                "bk": np.ascontiguousarray(bk.reshape(NJ, 128).T),
                "ident": np.eye(128, dtype=np.float32).astype(BF16),
            }
        )

    global _LAST_IN_MAPS, _LAST_NC
    _LAST_IN_MAPS = in_maps
    _LAST_NC = nc
    res = bass_utils.run_bass_kernel_spmd(nc, in_maps, core_ids=list(range(8)))
    outs = res.results

    out = np.empty((B, N, DM), np.float32)
    for b in range(B):
        o = outs[2 * b]["out"].astype(np.float32) + outs[2 * b + 1]["out"].astype(np.float32)
        full = np.empty((N, DM), np.float32)
        full[qidx[b]] = o[: len(qidx[b])] + bo[None, :]
        # reference semantics for fully-masked query rows: uniform attention
        uf = (x[b].mean(0) @ Wv) @ Wo + bo
        full[~mask_q[b]] = uf
        out[b] = full
    return out



# revision 51
# speedup vs baseline: 1.1957x; 1.0061x over previous
"""Trainium2 Bass kernel for LocalAttentionLayer.

Problem: B=4, N=2048, H=8 heads, D=64, DM=512 (f32)
  q/k/v = x @ W{q,k,v}; sim = scale * q k^T (per head); mask_k/mask_q -> big_neg;
  softmax over keys; out = (attn @ v) @ Wo + bo.

Sharding (8 cores): core = 2*b + g -> batch b (4-way) x head-group g (2-way,
4 heads each).  Each core computes its batch's projections for its 4 heads,
full attention for those heads, and a partial output projection with its
256-row slice of Wo.  Host sums the two partials per batch, adds bo, and
overwrites masked-q rows (reference semantics: fully-masked rows degenerate
to uniform attention = mean over all v rows, computable on host as
(mean_j x) @ Wv @ Wo + bo).

Key optimizations over the naive layout:
  - Masked-position compaction: only kept q rows (Pq) and kept k rows (Pk)
    are shipped/computed; host gathers inputs and scatters outputs.  Pq/Pk
    are runtime values (q rounded up to 32, k to 128); one program is
    compiled per (Pq, Pk) and cached.
  - The attention inner loop is software-pipelined and ACT(Exp)-paced: sim
    j+2 is issued before pv j, each chunk's normalize is deferred until
    after the next chunk's first sims, and all non-attention PE work
    (k/q/v projections for later chunks, the hp1 projections, the output
    projection) is streamed through a work queue popped between sims so the
    Exp engine never waits at a phase boundary.
  - Softmax denominator rides along as a ones-column in v (col 64 of each
    68-wide head block), so P@V and the denominators come out of the same
    accumulation; all matmuls are bf16 (fp8 was measured too lossy: >1e-2).
  - PSUM-bank discipline: every matmul output stays inside one 2KB bank;
    the Exp uses strided 3-D APs to skip the inter-head alignment gap.
  - Copies and normalize run on DVE explicitly; ACT only does Exp.
  - Each input tensor loads as one wide multi-dim DMA (the HWDGE setup is
    a serial ~630ns/DMA resource), ordered by first use, with xq/xk split
    once so chunk-0 compute starts ~3us in.
"""

import sys

if "/opt/trn_rl_repo" not in sys.path:
    sys.path.insert(0, "/opt/trn_rl_repo")

from collections import deque

import os

import ml_dtypes
import numpy as np

SLACK_MARGIN = float(os.environ.get("K_SLACK", "120"))
LAG_LIMIT = int(os.environ.get("K_LAG", "6"))
PT_BUFS = int(os.environ.get("K_PTBUFS", "28"))

BF16 = np.dtype(ml_dtypes.bfloat16)

B, N, H, D = 4, 2048, 8, 64
DM = H * D  # 512
G = 2  # head-group split across cores
CG = DM // G  # 256 channels per group
HPG = H // G  # 4 heads per group
MASK_BIAS = -1.0e5
SHIFT = -4.0  # logit shift: keeps exp() comfortably in range without row max

_NC_CACHE = {}


def _build_nc(Pq, Pk):
    from contextlib import ExitStack

    import concourse.mybir as mybir
    import concourse.tile as tile
    from concourse import bacc
    from concourse.bass import ts

    f32 = mybir.dt.float32
    bf16 = mybir.dt.bfloat16
    EXP = mybir.ActivationFunctionType.Exp

    NJ = Pk // 128  # j-tiles
    # output i-tiles (last may be a partial tile: Pq is a multiple of 32)
    OT = []
    off = 0
    while off < Pq:
        OT.append((off, min(128, Pq - off)))
        off += 128
    NI = len(OT)
    # attention i-chunks: full 512-wide (sim output = whole PSUM bank per
    # head) plus a small remainder chunk.  hp0 visits the small chunk FIRST
    # (less DMA before the first Exp), hp1 visits it LAST (short drain after
    # the final Exp).
    CHS = [512] * (Pq // 512) + ([Pq % 512] if Pq % 512 else [])
    COFF = [sum(CHS[:i]) for i in range(len(CHS))]
    CHUNKS = list(zip(COFF, CHS))  # (i0, ic)
    if int(os.environ.get("K_ORDER", "0")):
        ORDER = {
            0: sorted(CHUNKS, key=lambda t: t[1]),
            1: sorted(CHUNKS, key=lambda t: -t[1]),
        }
    else:
        ORDER = {0: list(CHUNKS), 1: list(CHUNKS)}
    ICK = min(512, Pk)  # first k-projection chunk (one PSUM bank wide)
    K_CHUNKS = []
    _o = ICK
    while _o < Pk:
        K_CHUNKS.append((_o, min(512, Pk - _o)))
        _o += 512

    nc = bacc.Bacc(None, target_bir_lowering=False, debug=False)

    with tile.TileContext(nc) as tc, ExitStack() as ctx:
        dram = ctx.enter_context(tc.tile_pool(name="dram", bufs=1, space="DRAM"))
        const = ctx.enter_context(tc.tile_pool(name="const", bufs=1))
        ptp = ctx.enter_context(tc.tile_pool(name="ptp", bufs=PT_BUFS))
        fop = ctx.enter_context(tc.tile_pool(name="fop", bufs=4))
        rrp = ctx.enter_context(tc.tile_pool(name="rrp", bufs=6))
        psim = ctx.enter_context(tc.tile_pool(name="psim", bufs=2, space="PSUM"))
        ppv = ctx.enter_context(tc.tile_pool(name="ppv", bufs=1, space="PSUM"))
        pfo = ctx.enter_context(tc.tile_pool(name="pfo", bufs=2, space="PSUM"))

        # ---- DRAM I/O ----
        xqT_d = dram.tile([DM, Pq], bf16, kind="ExternalInput", name="xqT", uniquify=False)
        xkT_d = dram.tile([DM, Pk], bf16, kind="ExternalInput", name="xkT", uniquify=False)
        wq_d = dram.tile([DM, CG], bf16, kind="ExternalInput", name="wq", uniquify=False)
        wk_d = dram.tile([DM, CG], bf16, kind="ExternalInput", name="wk", uniquify=False)
        wva_d = dram.tile([DM, CG], bf16, kind="ExternalInput", name="wva", uniquify=False)
        wo_d = dram.tile([CG, DM], bf16, kind="ExternalInput", name="wo", uniquify=False)
        bk_d = dram.tile([128, NJ], f32, kind="ExternalInput", name="bk", uniquify=False)
        id_d = dram.tile([128, 128], bf16, kind="ExternalInput", name="ident", uniquify=False)
        out_d = dram.tile([Pq, DM], bf16, kind="ExternalOutput", name="out", uniquify=False)

        # ---- SBUF persistents ----
        # The HWDGE + DMA engines are a serial resource (~630ns setup per
        # DMA), so each tensor loads as ONE wide DMA ([128, slices, cols]
        # APs), ordered by first use; xq/xk split once so chunk-0 arrives
        # early and compute starts ~3us in.
        xq_r = xqT_d.rearrange("(s p) i -> p s i", s=4, p=128)
        xk_r = xkT_d.rearrange("(s p) i -> p s i", s=4, p=128)
        wq_sb = const.tile([128, 4, CG], bf16, name="wq_sb")
        nc.sync.dma_start(out=wq_sb[:, :, :], in_=wq_d.rearrange("(s p) c -> p s c", s=4, p=128))
        xqT_sb = const.tile([128, 4, Pq], bf16, name="xqT_sb")
        nc.sync.dma_start(out=xqT_sb[:, :, 0 : CHS[0]], in_=xq_r[:, :, 0 : CHS[0]])
        wk_sb = const.tile([128, 4, CG], bf16, name="wk_sb")
        nc.sync.dma_start(out=wk_sb[:, :, :], in_=wk_d.rearrange("(s p) c -> p s c", s=4, p=128))
        bk_sb = const.tile_from(bk_d[:, :], name="bks")
        xkT_sb = const.tile([128, 4, Pk], bf16, name="xkT_sb")
        XK0 = min(256, ICK)
        nc.sync.dma_start(out=xkT_sb[:, :, 0:XK0], in_=xk_r[:, :, 0:XK0])
        nc.sync.dma_start(out=xkT_sb[:, :, XK0:ICK], in_=xk_r[:, :, XK0:ICK])
        nc.sync.dma_start(out=xkT_sb[:, :, ICK:Pk], in_=xk_r[:, :, ICK:Pk])
        wva_sb = const.tile([128, 4, CG], bf16, name="wva_sb")
        nc.sync.dma_start(out=wva_sb[:, :, :], in_=wva_d.rearrange("(s p) c -> p s c", s=4, p=128))
        nc.sync.dma_start(out=xqT_sb[:, :, CHS[0] : Pq], in_=xq_r[:, :, CHS[0] : Pq])
        id_sb = const.tile([128, 128], bf16, name="id_sb")
        nc.sync.dma_start(out=id_sb[:, :], in_=id_d[:, :])
        wo_sb = const.tile([128, 2, DM], bf16, name="wo_sb")
        nc.sync.dma_start(out=wo_sb[:, :, :], in_=wo_d.rearrange("(s p) c -> p s c", s=2, p=128))



        qT_sb = [const.tile([128, Pq], bf16, name=f"qT{hp}") for hp in range(2)]
        kT_sb = [const.tile([128, Pk], bf16, name=f"kT{hp}") for hp in range(2)]
        aT_sb = [const.tile([128, Pq], bf16, name=f"aT{hp}") for hp in range(2)]
        # va: per j-tile [128, HPG*65] bf16: 4 heads x (64 v-cols + ones col);
        # the ones columns are memset once up front and never overwritten
        va_sb = [const.tile([128, HPG, 65], bf16, name=f"va{j}") for j in range(NJ)]
        for j in range(NJ):
            nc.vector.memset(va_sb[j][:, :, 64:65], 1.0)
        if int(os.environ.get("K_DUMMY", "1")):
            # dummy Exp: pulls the ~1.3us ACT_TABLE_LOAD to t=0 instead of
            # just before the first real Exp
            dum = const.tile([1, 2], f32, name="dum")
            nc.vector.memset(dum[:, :], 0.0)
            nc.scalar.activation(dum[0:1, 1:2], dum[0:1, 0:1], EXP)
        NWARM = int(os.environ.get("K_WARM", "2"))
        if NWARM:
            # PE p-state warmup: ~3us of dummy matmuls from t=0 bring the
            # tensor engine to full clock while the first DMAs land, so the
            # preamble projections run at 2.4GHz instead of the cold clock
            wmw = const.tile([128, 128], bf16, name="wmw")
            nc.vector.memset(wmw[:, :], 0.0)
            wmx = const.tile([128, 512], bf16, name="wmx")
            nc.vector.memset(wmx[:, :], 0.0)
            for _ in range(NWARM):
                wm = psim.tile([128, 1024], f32, tag="sim", name="wm")
                for h in range(2):
                    nc.tensor.matmul(
                        wm[:, h * 512 : (h + 1) * 512],
                        wmw[:, :],
                        wmx[:, :],
                        start=True,
                        stop=True,
                    )

        # ---- projection / output helpers ----
        def qk_proj_group(w_sb, x_sb, dst, hp, off, width):
            """One chunk of a q/k projection: dst[:, off:off+width]."""
            ps = pfo.tile([128, 512], f32, tag="fo", name="qk_ps")
            for k in range(4):
                nc.tensor.matmul(
                    ps[:, 0:width],
                    w_sb[:, k, hp * 128 : (hp + 1) * 128],
                    x_sb[:, k, off : off + width],
                    start=(k == 0),
                    stop=(k == 3),
                )
            nc.vector.tensor_copy(dst[:, off : off + width], ps[:, 0:width])

        # emitted-coverage bookkeeping: sims may only be emitted once the
        # kT/qT columns they read have their producers emitted (dependency
        # tracking follows emission order); cov counts contiguous columns
        cov = {"k0": 0, "k1": 0, "q0": [], "q1": []}

        NSLOT_HP = len(CHS) * NJ  # slots per head-pair

        def push_qk(w_sb, x_sb, dst, hp, off, width, kind, deadline=10**9):
            def f():
                qk_proj_group(w_sb, x_sb, dst, hp, off, width)
                if kind[0] == "k":
                    cov[kind] = max(cov[kind], off + width)
                else:
                    cov[kind].append((off, off + width))

            push(QK_NS, f, deadline=deadline)

        def force_k(hp, need):
            while cov[f"k{hp}"] < need:
                assert work_q, f"cannot extend k{hp} coverage to {need}"
                _pop_one()

        def force_q(hp, i0, ic):
            def done():
                return any(a <= i0 and i0 + ic <= b for a, b in cov[f"q{hp}"])

            while not done():
                assert work_q, f"cannot cover q{hp} [{i0}:{i0 + ic}]"
                _pop_one()

        def v_proj(j):
            v_ps = pfo.tile([128, CG], f32, tag="fo", name="v_ps")
            for k in range(4):
                nc.tensor.matmul(
                    v_ps[:, :],
                    xkT_sb[:, k, ts(j, 128)],
                    wva_sb[:, k, :],
                    start=(k == 0),
                    stop=(k == 3),
                )
            nc.vector.tensor_copy(
                va_sb[j][:, :, 0:64],
                v_ps.rearrange("p (h c) -> p h c", h=HPG, c=64)[:, :, :],
            )

        # ---- attention: globally slot-scheduled ----
        # ACT is the binding engine now.  Each j slot emits its sim matmuls
        # and the Exp; deferred PE units (k/q/v projections, chunk epilogues
        # with the two-pass pv + output projection) pop from a FIFO under a
        # per-slot cost budget so PE fills the ACT-bound slack.
        work_q = deque()  # (est_cost_ns, emit_fn, is_pv, deadline_slot)
        budget = [0.0]
        pv_lag = [0]  # un-popped epilogue units; bounded by the pt ring depth
        PV_LAG_LIMIT = LAG_LIMIT
        cur_slot = [0]
        LOOKAHEAD = int(os.environ.get("K_LOOKAHEAD", "3"))

        def push(cost, fn, is_pv=False, deadline=10**9):
            work_q.append((cost, fn, is_pv, deadline))
            if is_pv:
                pv_lag[0] += 1

        def _pop_item(item):
            cost, fn, is_pv, _ = item
            budget[0] -= cost
            if is_pv:
                pv_lag[0] -= 1
            fn()

        def _pop_one():
            _pop_item(work_q.popleft())

        def pops():
            # deadline-near units pop regardless of budget, spread across
            # slots instead of bursting at a chunk boundary
            horizon = cur_slot[0] + LOOKAHEAD
            while True:
                hit = next((i for i, u in enumerate(work_q) if u[3] <= horizon), None)
                if hit is None:
                    break
                item = work_q[hit]
                del work_q[hit]
                _pop_item(item)
            while work_q and work_q[0][0] <= budget[0]:
                _pop_one()
            # an epilogue backlog deeper than the pt ring would corrupt the
            # ring: force-drain ahead of budget
            while pv_lag[0] > PV_LAG_LIMIT:
                _pop_one()


        V_NS = 4 * CG * 0.42
        QK_NS = 4 * 512 * 0.42
        OUT_NS = (2 * DM + DM) * 0.42

        held_pv = {}

        def mk_pv_partial(hp, h, io, ilen, i0, pts, js):
            def f():
                epi_popped[0] += 1
                pvt = ppv.tile([128, 65], f32, tag=f"pv{h}", name=f"pv{h}")
                held_pv[(io, h)] = pvt
                c0 = h * 512 + (io - i0)
                for j in range(js):
                    nc.tensor.matmul(
                        pvt[0:ilen, :],
                        pts[j][:, c0 : c0 + ilen],
                        va_sb[j][:, hp * 2 + h, :],
                        start=(j == 0),
                        stop=False,
                    )

            return f

        def mk_pv_finish(hp, h, io, ilen, i0, pts, js, a_sb, rr):
            def f():
                epi_popped[0] += 1
                pvt = held_pv.pop((io, h))
                c0 = h * 512 + (io - i0)
                for j in range(js, NJ):
                    nc.tensor.matmul(
                        pvt[0:ilen, :],
                        pts[j][:, c0 : c0 + ilen],
                        va_sb[j][:, hp * 2 + h, :],
                        start=False,
                        stop=(j == NJ - 1),
                    )
                nc.vector.reciprocal(rr[0:ilen, h : h + 1], pvt[0:ilen, 64:65])
                nc.vector.tensor_scalar_mul(
                    a_sb[0:ilen, h * 64 : (h + 1) * 64],
                    pvt[0:ilen, 0:64],
                    rr[0:ilen, h : h + 1],
                )

            return f

        def mk_pvh(hp, h, io, ilen, i0, pts, a_sb, rr):
            """Two-pass pv for one (i-tile, head): replay the chunk's stored
            pt tiles as STATIONARY operands (ldweights are free) into a
            [128 q, 65] accumulator -- 65 output columns per j instead of ic,
            the softmax denominator riding along in column 64 -- then
            normalize with a per-partition reciprocal."""

            def f():
                epi_popped[0] += 1
                pvt = ppv.tile([128, 65], f32, tag=f"pv{h}", name=f"pv{h}")
                c0 = h * 512 + (io - i0)
                for j in range(NJ):
                    nc.tensor.matmul(
                        pvt[0:ilen, :],
                        pts[j][:, c0 : c0 + ilen],
                        va_sb[j][:, hp * 2 + h, :],
                        start=(j == 0),
                        stop=(j == NJ - 1),
                    )
                nc.vector.reciprocal(rr[0:ilen, h : h + 1], pvt[0:ilen, 64:65])
                nc.vector.tensor_scalar_mul(
                    a_sb[0:ilen, h * 64 : (h + 1) * 64],
                    pvt[0:ilen, 0:64],
                    rr[0:ilen, h : h + 1],
                )

            return f

        def mk_tx(hp, io, ilen, a_sb):
            """Flip the normalized [q, d] tile to [d, q] through the PE with
            an identity, then (hp1) the output projection for the i-tile."""

            def f():
                epi_popped[0] += 1
                tp = pfo.tile([128, 128], f32, tag="fo", name="tp")
                nc.tensor.matmul(
                    tp[:, 0:ilen],
                    a_sb[0:ilen, :],
                    id_sb[0:ilen, 0:ilen],
                    start=True,
                    stop=True,
                )
                nc.vector.tensor_copy(aT_sb[hp][:, io : io + ilen], tp[:, 0:ilen])
                if hp == 1:
                    out_proj_of(io, ilen)

            return f

        def out_proj_of(io, ilen):
            fo = pfo.tile([128, 512], f32, tag="fo", name="fo_ps")
            for c in range(2):
                nc.tensor.matmul(
                    fo[0:ilen, :],
                    aT_sb[c][:, io : io + ilen],
                    wo_sb[:, c, :],
                    start=(c == 0),
                    stop=(c == 1),
                )
            fo_sb = fop.tile([128, 512], bf16, tag="fos", name="fo_sb")
            nc.vector.tensor_copy(fo_sb[0:ilen, :], fo[0:ilen, :])
            nc.sync.dma_start(out=out_d[io : io + ilen, :], in_=fo_sb[0:ilen, :])

        PVH_NS = NJ * 65 * 0.42
        TX_NS = (128 + 1024) * 0.42
        epi_pushed = [0]
        epi_popped = [0]
        chunk_marks = []  # epi_pushed watermark at each chunk start

        def attention(hp, i0, ic, last=False):
            # pt-ring safety: every epilogue reading pt tiles from two chunks
            # ago must be EMITTED before this chunk's sims reuse those ring
            # slots (dependency tracking follows emission order)
            chunk_marks.append(epi_pushed[0])
            if last and int(os.environ.get("K_DRAIN", "1")):
                # the final chunk's own epilogue is the only drain after the
                # last Exp -- flush every earlier epilogue into this chunk's
                # slot stream instead of the tail
                while epi_popped[0] < epi_pushed[0]:
                    _pop_one()
            elif len(chunk_marks) >= 3:
                need = chunk_marks[-2]
                while epi_popped[0] < need:
                    _pop_one()
            slot_slack = 2 * ic * 0.4167 + SLACK_MARGIN
            if hp == 1:
                slot_slack += float(os.environ.get("K_SLACK1", "0"))

            def emit_sim(j):
                # head blocks bank-aligned at h*512: one matmul output window
                # per PSUM bank
                sm = psim.tile([128, 1024], f32, tag="sim", name="sm")
                for h in range(2):
                    hs = slice(h * 64, (h + 1) * 64)
                    nc.tensor.matmul(
                        sm[:, h * 512 : h * 512 + ic],
                        kT_sb[hp][hs, ts(j, 128)],
                        qT_sb[hp][hs, i0 : i0 + ic],
                        start=True,
                        stop=True,
                    )
                ptv = ptp.tile([128, 2 * 512], bf16, tag="pt", name="pt")
                sm_v = sm.rearrange("p (h c) -> p h c", h=2, c=512)[:, :, 0:ic]
                pt_v = ptv.rearrange("p (h c) -> p h c", h=2, c=512)[:, :, 0:ic]
                nc.scalar.activation(pt_v, sm_v, EXP, bias=bk_sb[:, j : j + 1], scale=1.0)
                return ptv

            force_q(hp, i0, ic)
            force_k(hp, 128)
            JS = NJ - 2
            pts = [emit_sim(0)]
            for j in range(NJ):
                cur_slot[0] += 1
                if j + 1 < NJ:
                    force_k(hp, min((j + 2) * 128, Pk))
                    pts.append(emit_sim(j + 1))
                if hp == 0 and i0 == ORDER[0][0][0] and j + 2 < NJ:
                    push(V_NS, lambda j2=j + 2: v_proj(j2))
                if last and j == JS and ic >= 128:
                    # head-start the final i-tile's pv so only two j-tiles
                    # remain after the last Exp
                    for h in range(2):
                        push(JS * 65 * 0.42, mk_pv_partial(hp, h, i0, 128, i0, pts, JS), is_pv=True)
                        epi_pushed[0] += 1
                budget[0] += slot_slack
                pops()
            # chunk epilogue units (consume this chunk's pt ring slots); they
            # pop inside the following chunks' slot slack.  is_pv bounds how
            # far they may lag so the pt ring never wraps onto unread tiles.
            off = i0
            COARSE = int(os.environ.get("K_COARSE", "1"))
            while off < i0 + ic:
                ilen = min(128, i0 + ic - off)
                a_sb = rrp.tile([128, 128], bf16, tag="an", name="an")
                rr = rrp.tile([128, 2], f32, tag="rr", name="rr")
                if last and off == i0 and ic >= 128:
                    # finish the head-started tile: 2 j-tiles + normalize + tx
                    def fin(hp=hp, off=off, i0=i0, a_sb=a_sb, rr=rr, pts=pts, JS=JS):
                        # inner calls bump epi_popped once each (3 total);
                        # net for this single pushed unit must be +1 -> -2
                        epi_popped[0] -= 2
                        mk_pv_finish(hp, 0, off, 128, i0, pts, JS, a_sb, rr)()
                        mk_pv_finish(hp, 1, off, 128, i0, pts, JS, a_sb, rr)()
                        mk_tx(hp, off, 128, a_sb)()

                    push((2 * (NJ - JS) * 65 + 128 + 1024) * 0.42, fin, is_pv=True)
                    epi_pushed[0] += 1
                    off += 128
                    continue
                if COARSE:
                    u0 = mk_pvh(hp, 0, off, ilen, i0, pts, a_sb, rr)
                    u1 = mk_pvh(hp, 1, off, ilen, i0, pts, a_sb, rr)
                    u2 = mk_tx(hp, off, ilen, a_sb)

                    def fused(u0=u0, u1=u1, u2=u2):
                        u0(), u1(), u2()
                        epi_popped[0] += 2  # fused units count as three pops

                    push(2 * PVH_NS + TX_NS, fused, is_pv=True)
                    epi_pushed[0] += 1
                else:
                    for h in range(2):
                        push(PVH_NS, mk_pvh(hp, h, off, ilen, i0, pts, a_sb, rr), is_pv=True)
                        epi_pushed[0] += 1
                    push(TX_NS, mk_tx(hp, off, ilen, a_sb), is_pv=True)
                    epi_pushed[0] += 1
                off += ilen

        # ---- program ----
        first0 = ORDER[0][0]
        qk_proj_group(wq_sb, xqT_sb, qT_sb[0], 0, first0[0], first0[1])
        cov["q0"].append((first0[0], first0[0] + first0[1]))
        qk_proj_group(wk_sb, xkT_sb, kT_sb[0], 0, 0, XK0)
        qk_proj_group(wk_sb, xkT_sb, kT_sb[0], 0, XK0, ICK - XK0)
        cov["k0"] = ICK
        push(V_NS, lambda: v_proj(0))
        push(V_NS, lambda: v_proj(1))

        # seed deferred work: k/q chunks first (hard deadlines: k for this
        # chunk's own sims, q before chunk 1), v next (needed only when the
        # corresponding pv pops, well after slot j)
        for a, w in K_CHUNKS:
            # k columns [a:b] are read by slot a//128 of every chunk; the
            # binding one is the first chunk of the head-pair
            push_qk(wk_sb, xkT_sb, kT_sb[0], 0, a, w, "k0",
                    deadline=max(0, a // 128 - 1))
        for ni, (i0c, icc) in enumerate(ORDER[0][1:]):
            push_qk(wq_sb, xqT_sb, qT_sb[0], 0, i0c, icc, "q0",
                    deadline=(ni + 1) * NJ - 1)
        for hp in range(2):
            for ci, (i0c, icc) in enumerate(ORDER[hp]):
                if (hp, ci) == (0, 1):
                    # hp1 projections: pushed here so they pop before hp1
                    for ni, (i2, c2) in enumerate(ORDER[1]):
                        push_qk(wq_sb, xqT_sb, qT_sb[1], 1, i2, c2, "q1",
                                deadline=NSLOT_HP + ni * NJ - 1)
                    for a, w in [(0, ICK)] + K_CHUNKS:
                        push_qk(wk_sb, xkT_sb, kT_sb[1], 1, a, w, "k1",
                                deadline=NSLOT_HP + max(0, a // 128 - 1))
                attention(hp, i0c, icc, last=(hp == 1 and ci == len(ORDER[1]) - 1))
        budget[0] = 1e12
        pops()
        while work_q:
            _pop_one()

    nc.compile()
    return nc


def _get_nc(Pq=1696, Pk=1664):
    key = (Pq, Pk)
    if key not in _NC_CACHE:
        _NC_CACHE[key] = _build_nc(Pq, Pk)
    return _NC_CACHE[key]


def _roundup(n, m):
    return ((n + m - 1) // m) * m


def kernel(x, Wq, Wk, Wv, Wo, bo, mask_k, mask_q):
    from concourse import bass_utils

    x = np.asarray(x, np.float32)
    Wq = np.asarray(Wq, np.float32)
    Wk = np.asarray(Wk, np.float32)
    Wv = np.asarray(Wv, np.float32)
    Wo = np.asarray(Wo, np.float32)
    bo = np.asarray(bo, np.float32)
    mask_k = np.asarray(mask_k)
    mask_q = np.asarray(mask_q)

    qidx = [np.nonzero(mask_q[b])[0] for b in range(B)]
    kidx = [np.nonzero(mask_k[b])[0] for b in range(B)]
    # compaction pads; >=1024 keeps the chunk layout simple
    Pq = max(_roundup(max(len(i) for i in qidx), 32), 1024)
    Pk = max(_roundup(max(len(i) for i in kidx), 128), 1024)
    NJ = Pk // 128

    nc = _get_nc(Pq, Pk)
    scale = float(D) ** -0.5

    in_maps = []
    for core in range(8):
        b, g = core // 2, core % 2
        cs = slice(g * CG, (g + 1) * CG)
        qi, ki = qidx[b], kidx[b]
        xq = np.zeros((Pq, DM), np.float32)
        xq[: len(qi)] = x[b][qi]
        xk = np.zeros((Pk, DM), np.float32)
        xk[: len(ki)] = x[b][ki]
        bk = np.full((Pk,), MASK_BIAS, np.float32)
        bk[: len(ki)] = SHIFT
        in_maps.append(
            {
                "xqT": np.ascontiguousarray(xq.T).astype(BF16),
                "xkT": np.ascontiguousarray(xk.T).astype(BF16),
                "wq": np.ascontiguousarray(Wq[:, cs] * scale).astype(BF16),
                "wk": np.ascontiguousarray(Wk[:, cs]).astype(BF16),
                "wva": np.ascontiguousarray(Wv[:, cs]).astype(BF16),
                "wo": np.ascontiguousarray(Wo[cs, :]).astype(BF16),
                "bk": np.ascontiguousarray(bk.reshape(NJ, 128).T),
                "ident": np.eye(128, dtype=np.float32).astype(BF16),
            }
        )

    global _LAST_IN_MAPS, _LAST_NC
    _LAST_IN_MAPS = in_maps
    _LAST_NC = nc
    res = bass_utils.run_bass_kernel_spmd(nc, in_maps, core_ids=list(range(8)))
    outs = res.results

    out = np.empty((B, N, DM), np.float32)
    for b in range(B):
        o = outs[2 * b]["out"].astype(np.float32) + outs[2 * b + 1]["out"].astype(np.float32)
        full = np.empty((N, DM), np.float32)
        full[qidx[b]] = o[: len(qidx[b])] + bo[None, :]
        # reference semantics for fully-masked query rows: uniform attention
        uf = (x[b].mean(0) @ Wv) @ Wo + bo
        full[~mask_q[b]] = uf
        out[b] = full
    return out

